# revision 1
# baseline (speedup 1.0000x reference)
"""MoE layer (top-2 routing, 8 experts) on 8 Trainium2 NeuronCores.

Sharding: token-parallel. Each core takes a contiguous shard of 1024 tokens
(of T=8192) and runs them through all 8 experts densely, weighting each
expert's output by the on-device-computed combine weights (softmax over the
top-2 router logits; zero elsewhere). No cross-core communication is needed:
each core produces its own 1024-row slice of the output, and the host only
concatenates the slices.

Router runs in fp32 (top-2 selection must match the fp32 reference exactly;
measured logit margins ~5.7e-5 far exceed fp32 matmul rounding). FFN matmuls
run in bf16 with fp32 PSUM accumulation.
"""

import sys, os

for _p in ("/root/.axon_site", "/root/.axon_site/_ro/trn_rl_repo",
           "/root/.axon_site/_ro/pypackages", "/opt/trn_rl_repo"):
    if os.path.isdir(_p) and _p not in sys.path:
        sys.path.append(_p)

import numpy as np
import ml_dtypes

BF16 = ml_dtypes.bfloat16

T, DIM, E, K, H = 8192, 1024, 8, 2, 4096
N_CORES = 8
TPC = T // N_CORES          # tokens per core = 1024
NTB = TPC // 128            # 8 token tiles per core
ND = DIM // 128             # 8 d-chunks
NH = H // 128               # 32 h-chunks
NT = 512                    # matmul moving-dim (tokens per psum group)
NNT = TPC // NT             # 2 groups per core

_compiled = None


def _build():
    from concourse import bass, bacc, tile, mybir
    from concourse.masks import make_identity

    dt = mybir.dt
    nc = bacc.Bacc("TRN2", target_bir_lowering=False, debug=False,
                   num_devices=N_CORES)

    xtf = nc.dram_tensor("xtf", [NTB, ND, 128, 128], dt.float32, kind="ExternalInput").ap()
    xbt = nc.dram_tensor("xbt", [NTB, ND, 128, 128], dt.bfloat16, kind="ExternalInput").ap()
    wr = nc.dram_tensor("wr", [DIM, E], dt.float32, kind="ExternalInput").ap()
    w1d = nc.dram_tensor("w1d", [E, NH, 128, ND, 128], dt.bfloat16, kind="ExternalInput").ap()
    w2d = nc.dram_tensor("w2d", [E, ND, 128, NH, 128], dt.bfloat16, kind="ExternalInput").ap()
    b1d = nc.dram_tensor("b1d", [128, E, NH], dt.float32, kind="ExternalInput").ap()
    b2d = nc.dram_tensor("b2d", [128, E, ND], dt.float32, kind="ExternalInput").ap()
    out = nc.dram_tensor("out_shard", [TPC, DIM], dt.float32, kind="ExternalOutput").ap()

    with tile.TileContext(nc) as tc:
        with tc.tile_pool(name="const", bufs=1) as const, \
             tc.tile_pool(name="resident", bufs=1) as res, \
             tc.tile_pool(name="w1p", bufs=3) as w1p, \
             tc.tile_pool(name="w2p", bufs=2) as w2p, \
             tc.tile_pool(name="xrp", bufs=3) as xrp, \
             tc.tile_pool(name="vec", bufs=2) as vec, \
             tc.tile_pool(name="pmm", bufs=4, space="PSUM") as pmm, \
             tc.tile_pool(name="ptr", bufs=2, space="PSUM") as ptr:

            ident = const.tile([128, 128], dt.float32)
            make_identity(nc, ident[:])
            identb = const.tile([128, 128], dt.bfloat16)
            nc.vector.tensor_copy(identb[:], ident[:])

            # resident tensors
            xall = res.tile([128, ND, TPC], dt.bfloat16)   # x^T bf16, 16KB/part
            acc = res.tile([128, NTB, DIM], dt.float32)    # output accum, 32KB/part
            hT = res.tile([128, NH, TPC], dt.bfloat16)     # hidden, 64KB/part
            cmb = res.tile([128, NTB, E], dt.float32)      # combine weights
            lg = res.tile([128, NTB, E], dt.float32)       # logits
            mx = res.tile([128, NTB, 8], dt.float32)       # sorted top-8
            wr_sb = const.tile([128, ND, E], dt.float32)
            b1sb = const.tile([128, E, NH], dt.float32)
            b2sb = const.tile([128, E, ND], dt.float32)

            nc.vector.memset(acc[:], 0.0)
            nc.sync.dma_start(b1sb[:], b1d[:])
            nc.sync.dma_start(b2sb[:], b2d[:])
            for dc in range(ND):
                nc.sync.dma_start(wr_sb[:, dc, :], wr[dc * 128:(dc + 1) * 128, :])
            # load x^T bf16 (FFN rhs), resident
            for tb in range(NTB):
                for dc in range(ND):
                    nc.sync.dma_start(xall[:, dc, tb * 128:(tb + 1) * 128],
                                      xbt[tb, dc])

            # ---- router (fp32) ----
            def emit_router():
              for tb in range(NTB):
                xt = xrp.tile([128, ND, 128], dt.float32, tag="xt", name="xt")
                nc.sync.dma_start(xt[:], xtf[tb].transpose([1, 0, 2]))
                ps = ptr.tile([128, E], dt.float32, name=f"psr_{tb}", tag="psr")
                for dc in range(ND):
                    nc.tensor.matmul(ps[:], lhsT=xt[:, dc, :], rhs=wr_sb[:, dc, :],
                                     start=(dc == 0), stop=(dc == ND - 1))
                nc.scalar.copy(lg[:, tb, :], ps[:])
                nc.vector.max(mx[:, tb, :], lg[:, tb, :])
              l1 = mx[:, :, 0]                       # [128, NTB] strided
              l2 = mx[:, :, 1]
              d12 = vec.tile([128, NTB], dt.float32, name="d12")
              p1 = vec.tile([128, NTB], dt.float32, name="p1")
              nc.vector.tensor_sub(d12[:], l1, l2)
              nc.scalar.activation(p1[:], d12[:],
                                   bass.mybir.ActivationFunctionType.Sigmoid)
              # cmb[:, tb, e] = (lg==l1)*p1 + (lg==l2)*(1-p1)
              m1 = vec.tile([128, NTB, E], dt.float32, name="m1")
              m2 = vec.tile([128, NTB, E], dt.float32, name="m2")
              l1b = l1.unsqueeze(2).to_broadcast([128, NTB, E])
              l2b = l2.unsqueeze(2).to_broadcast([128, NTB, E])
              p1b = p1[:].unsqueeze(2).to_broadcast([128, NTB, E])
              nc.vector.tensor_tensor(m1[:], lg[:], l1b, mybir.AluOpType.is_equal)
              nc.vector.tensor_tensor(m2[:], lg[:], l2b, mybir.AluOpType.is_equal)
              nc.vector.tensor_tensor(m1[:], m1[:], p1b, mybir.AluOpType.mult)
              t2 = vec.tile([128, NTB, E], dt.float32, name="t2")
              nc.vector.tensor_tensor(t2[:], m2[:], p1b, mybir.AluOpType.mult)
              nc.vector.tensor_sub(m2[:], m2[:], t2[:])
              nc.vector.tensor_add(cmb[:], m1[:], m2[:])

            # ---- FFN over experts ----
            def emit_l1(e):
                for hc in range(NH):
                    w1t = w1p.tile([128, ND, 128], dt.bfloat16, tag="w1t")
                    nc.sync.dma_start(w1t[:], w1d[e, hc])
                    for nt in range(NNT):
                        ps = pmm.tile([128, NT], dt.float32, name=f"ps1_{e}_{hc}_{nt}", tag="ps")
                        sl = slice(nt * NT, (nt + 1) * NT)
                        for dc in range(ND):
                            nc.tensor.matmul(ps[:], lhsT=w1t[:, dc, :],
                                             rhs=xall[:, dc, sl],
                                             start=(dc == 0), stop=(dc == ND - 1))
                        nc.scalar.activation(hT[:, hc, sl], ps[:],
                                             bass.mybir.ActivationFunctionType.Gelu,
                                             bias=b1sb[:, e, hc:hc + 1])
            # layer 2 + transpose + weighted accumulate
            def emit_l2(e):
                for dc in range(ND):
                    w2t = w2p.tile([128, NH, 128], dt.bfloat16, tag="w2t")
                    nc.sync.dma_start(w2t[:], w2d[e, dc])
                    for nt in range(NNT):
                        ps = pmm.tile([128, NT], dt.float32, name=f"ps2_{e}_{dc}_{nt}", tag="ps")
                        sl = slice(nt * NT, (nt + 1) * NT)
                        for hc in range(NH):
                            nc.tensor.matmul(ps[:], lhsT=w2t[:, hc, :],
                                             rhs=hT[:, hc, sl],
                                             start=(hc == 0), stop=(hc == NH - 1))
                        yt = vec.tile([128, NT], dt.bfloat16, tag="yt")
                        nc.scalar.activation(yt[:], ps[:],
                                             bass.mybir.ActivationFunctionType.Identity,
                                             bias=b2sb[:, e, dc:dc + 1])
                        for tc_ in range(NT // 128):
                            tb = nt * (NT // 128) + tc_
                            pt = ptr.tile([128, 128], dt.bfloat16,
                                          name=f"pt_{e}_{dc}_{nt}_{tc_}", tag="pt")
                            nc.tensor.transpose(
                                pt[:], yt[:, tc_ * 128:(tc_ + 1) * 128], identb[:])
                            a_sl = acc[:, tb, dc * 128:(dc + 1) * 128]
                            nc.vector.scalar_tensor_tensor(
                                a_sl, pt[:], cmb[:, tb, e:e + 1], a_sl,
                                op0=mybir.AluOpType.mult,
                                op1=mybir.AluOpType.add)

            emit_l1(0)
            emit_router()
            emit_l2(0)
            for e in range(1, E):
                emit_l1(e)
                emit_l2(e)

            for tb in range(NTB):
                nc.sync.dma_start(out[tb * 128:(tb + 1) * 128, :], acc[:, tb, :])

    nc.compile()
    return nc


def _prep_inputs(x, Wr, W1, b1, W2, b2):
    x = np.ascontiguousarray(np.asarray(x, np.float32)).reshape(T, DIM)
    Wr = np.ascontiguousarray(np.asarray(Wr, np.float32))
    W1 = np.asarray(W1, np.float32)
    b1 = np.asarray(b1, np.float32)
    W2 = np.asarray(W2, np.float32)
    b2 = np.asarray(b2, np.float32)

    w1d = np.ascontiguousarray(
        W1.astype(BF16).reshape(E, ND, 128, NH, 128).transpose(0, 3, 2, 1, 4))
    w2d = np.ascontiguousarray(
        W2.astype(BF16).reshape(E, NH, 128, ND, 128).transpose(0, 3, 2, 1, 4))
    b1d = np.ascontiguousarray(b1.reshape(E, NH, 128).transpose(2, 0, 1))
    b2d = np.ascontiguousarray(b2.reshape(E, ND, 128).transpose(2, 0, 1))

    in_maps = []
    for c in range(N_CORES):
        xs = x[c * TPC:(c + 1) * TPC]                      # [1024, 1024]
        # [NTB, ND, 128 d, 128 t] tiles of x^T
        xt = np.ascontiguousarray(
            xs.reshape(NTB, 128, ND, 128).transpose(0, 2, 3, 1))
        in_maps.append({
            "xtf": xt,
            "xbt": xt.astype(BF16),
            "wr": Wr,
            "w1d": w1d,
            "w2d": w2d,
            "b1d": b1d,
            "b2d": b2d,
        })
    return in_maps


def kernel(x, Wr, W1, b1, W2, b2, _profile=None):
    global _compiled
    from concourse.bass_utils import run_bass_kernel_spmd

    if _compiled is None:
        _compiled = _build()
    nc = _compiled
    in_maps = _prep_inputs(x, Wr, W1, b1, W2, b2)
    kwargs = {}
    if _profile:
        kwargs = dict(trace=True, tmpdir=_profile)
    res = run_bass_kernel_spmd(nc, in_maps, core_ids=list(range(N_CORES)), **kwargs)
    shards = [res.results[c]["out_shard"] for c in range(N_CORES)]
    full = np.concatenate(shards, axis=0).reshape(4, 2048, DIM).astype(np.float32)
    if _profile:
        return full, res
    return full



# revision 2
# speedup vs baseline: 1.0060x; 1.0060x over previous
"""MoE layer (top-2 routing, 8 experts) on 8 Trainium2 NeuronCores.

Sharding: expert-parallel (per the sharding hint). The host computes the
router (x @ Wr, top-2, softmax -- 134 MFLOP, 0.1% of the FFN work) and
shards tokens by expert assignment: core e receives the tokens routed to
expert e (zero-padded to capacity C=2176) plus expert e's weights. Each
core runs only its own expert's FFN in two token passes (1152 + 1024) so
the hidden activations fit SBUF. The host applies the top-2 combine
weights during unshard (each token's output is a weighted sum of exactly
two expert outputs -- a pure gather). Tokens beyond capacity (6 of 16384
for this input) are computed on host in exact fp32.

Device work per core: y^T = W2^T gelu(W1^T x_g^T + b1) + b2 over C
tokens in bf16 with fp32 PSUM accumulation: ~465 us of tensor-engine
work at 78.6 TF/s. DMA order is tuned so the first matmul starts ~7 us
in: w1 tile 0 and the first 512-token x slice are prefetched ahead of
the bulk x/weight streams.

Router runs in fp32 on host (top-2 selection matches the fp32 reference:
measured min margin between 2nd and 3rd logit is 5.7e-5, far above fp32
matmul rounding).
"""

import sys, os

for _p in ("/root/.axon_site", "/root/.axon_site/_ro/trn_rl_repo",
           "/root/.axon_site/_ro/pypackages", "/opt/trn_rl_repo"):
    if os.path.isdir(_p) and _p not in sys.path:
        sys.path.append(_p)

import numpy as np
import ml_dtypes

BF16 = ml_dtypes.bfloat16

T, DIM, E, K, H = 8192, 1024, 8, 2, 4096
N_CORES = 8
ND = DIM // 128             # 8 d-chunks
NH = H // 128               # 32 h-chunks
C = 2176                    # token capacity per core
PASSES = (1152, 1024)       # token split per pass (hT for one pass fits SBUF)
assert sum(PASSES) == C


def _groups(length):
    out = []
    off = 0
    while off < length:
        g = min(512, length - off)
        out.append((off, g))
        off += g
    return out


_compiled = None


def _build():
    from concourse import bass, bacc, tile, mybir

    dt = mybir.dt
    nc = bacc.Bacc("TRN2", target_bir_lowering=False, debug=False,
                   num_devices=N_CORES)

    # x^T in partition-major layout [128_d, ND, tokens], pre-split so each
    # load is one DMA with large contiguous DRAM lines (8-21KB/partition)
    XS0, XS1 = 512, PASSES[0]
    xg0 = nc.dram_tensor("xg0", [128, ND, XS0], dt.bfloat16, kind="ExternalInput").ap()
    xg1 = nc.dram_tensor("xg1", [128, ND, XS1 - XS0], dt.bfloat16, kind="ExternalInput").ap()
    xg2 = nc.dram_tensor("xg2", [128, ND, C - XS1], dt.bfloat16, kind="ExternalInput").ap()
    w1d = nc.dram_tensor("w1d", [NH, 128, ND, 128], dt.bfloat16, kind="ExternalInput").ap()
    w2d = nc.dram_tensor("w2d", [ND, 128, NH, 128], dt.bfloat16, kind="ExternalInput").ap()
    b1d = nc.dram_tensor("b1d", [128, NH], dt.float32, kind="ExternalInput").ap()
    b2d = nc.dram_tensor("b2d", [128, ND], dt.float32, kind="ExternalInput").ap()
    out = nc.dram_tensor("yT", [ND, 128, C], dt.bfloat16, kind="ExternalOutput").ap()

    with tile.TileContext(nc) as tc:
        with tc.tile_pool(name="const", bufs=1) as const, \
             tc.tile_pool(name="res", bufs=1) as res, \
             tc.tile_pool(name="w1p", bufs=3) as w1p, \
             tc.tile_pool(name="w2p", bufs=2) as w2p, \
             tc.tile_pool(name="ytp", bufs=4) as ytp, \
             tc.tile_pool(name="pmm", bufs=4, space="PSUM") as pmm:

            b1sb = const.tile([128, NH], dt.float32)
            b2sb = const.tile([128, ND], dt.float32)
            xall = res.tile([128, ND, C], dt.bfloat16)   # resident x^T
            hT = res.tile([128, NH, max(PASSES)], dt.bfloat16)

            # DMA queue assignment: w1 streams on sync (its only big job,
            # so w1 tiles are never delayed); w2 + half the outputs on
            # gpsimd; x is split across all three DMA-capable queues for
            # the fastest possible first-group load; outputs alternate
            # sync/gpsimd so the final drain uses two rings. Launches cost
            # ~0.6us each, so x uses a few large strided DMAs.
            w1_first = w1p.tile([128, ND, 128], dt.bfloat16, tag="w1t", name="w1f")
            nc.sync.dma_start(w1_first[:], w1d[0])
            nc.sync.dma_start(b1sb[:], b1d[:])
            nc.sync.dma_start(b2sb[:], b2d[:])
            # x loads: consumption order, parallel queues, contiguous src
            nc.gpsimd.dma_start(xall[:, :, 0:XS0], xg0[:])
            nc.scalar.dma_start(xall[:, :, XS0:XS1], xg1[:])
            nc.gpsimd.dma_start(xall[:, :, XS1:C], xg2[:])

            # PE warm-up during the x wait: ramps the tensor-engine p-state
            # so the first real chains run at full clock. Inputs: the already
            # -loaded w1 tile; output psum is never read.
            warm = pmm.tile([128, 512], dt.float32, name="warm")
            for i in range(16):
                nc.tensor.matmul(warm[:, :128], lhsT=w1_first[:, 0, :],
                                 rhs=w1_first[:, 1, :],
                                 start=(i == 0), stop=(i == 15))

            off = 0
            for plen in PASSES:
                # ---- layer 1: hT = gelu(W1^T x^T + b1) ----
                for hc in range(NH):
                    if off == 0 and hc == 0:
                        w1t = w1_first
                    else:
                        w1t = w1p.tile([128, ND, 128], dt.bfloat16, tag="w1t")
                        nc.sync.dma_start(w1t[:], w1d[hc])
                    for goff, glen in _groups(plen):
                        ps = pmm.tile([128, 512], dt.float32, tag="ps")
                        for dc in range(ND):
                            nc.tensor.matmul(ps[:, :glen], lhsT=w1t[:, dc, :],
                                             rhs=xall[:, dc, off + goff:off + goff + glen],
                                             start=(dc == 0), stop=(dc == ND - 1))
                        nc.scalar.activation(hT[:, hc, goff:goff + glen], ps[:, :glen],
                                             bass.mybir.ActivationFunctionType.Gelu,
                                             bias=b1sb[:, hc:hc + 1])
                # ---- layer 2: y^T = W2^T hT + b2 ----
                for dc in range(ND):
                    w2t = w2p.tile([128, NH, 128], dt.bfloat16, tag="w2t")
                    nc.gpsimd.dma_start(w2t[:], w2d[dc])
                    for gi, (goff, glen) in enumerate(_groups(plen)):
                        ps = pmm.tile([128, 512], dt.float32, tag="ps")
                        for hc in range(NH):
                            nc.tensor.matmul(ps[:, :glen], lhsT=w2t[:, hc, :],
                                             rhs=hT[:, hc, goff:goff + glen],
                                             start=(hc == 0), stop=(hc == NH - 1))
                        yt = ytp.tile([128, 512], dt.bfloat16, tag="yt")
                        nc.scalar.activation(yt[:, :glen], ps[:, :glen],
                                             bass.mybir.ActivationFunctionType.Identity,
                                             bias=b2sb[:, dc:dc + 1])
                        oeng = nc.gpsimd if (dc + gi) % 2 else nc.sync
                        oeng.dma_start(out[dc, :, off + goff:off + goff + glen],
                                       yt[:, :glen])
                off += plen

    nc.compile()
    return nc


def _route(x_flat, Wr):
    logits = x_flat.astype(np.float32) @ Wr.astype(np.float32)     # [T, E]
    order = np.argsort(-logits, axis=1)
    top2 = order[:, :K]                                            # [T, 2]
    l = np.take_along_axis(logits, top2, axis=1).astype(np.float64)
    m = l.max(axis=1, keepdims=True)
    p = np.exp(l - m)
    w = (p / p.sum(axis=1, keepdims=True)).astype(np.float32)      # [T, 2]
    return top2, w


def _gelu_exact(v):
    # exact gelu via erf, fp64 for host-computed overflow tokens
    from math import sqrt
    try:
        from scipy.special import erf
        return v * 0.5 * (1.0 + erf(v / sqrt(2.0)))
    except ImportError:
        from numpy import vectorize
        import math
        return v * 0.5 * (1.0 + np.vectorize(math.erf)(v / sqrt(2.0)))


def _prep(x, Wr, W1, b1, W2, b2):
    x_flat = np.ascontiguousarray(np.asarray(x, np.float32)).reshape(T, DIM)
    top2, w = _route(x_flat, np.asarray(Wr, np.float32))

    tok_lists = [np.where((top2 == e).any(axis=1))[0] for e in range(E)]

    # position of each (token, slot) pair inside its expert's gathered batch
    pos = np.zeros((T, K), dtype=np.int64)
    for e in range(E):
        for k in range(K):
            sel = top2[:, k] == e
            pos[sel, k] = np.searchsorted(tok_lists[e], np.where(sel)[0])

    W1 = np.asarray(W1, np.float32)
    W2 = np.asarray(W2, np.float32)
    b1 = np.asarray(b1, np.float32)
    b2 = np.asarray(b2, np.float32)

    in_maps = []
    overflow = []                                # (expert, token_ids) beyond capacity
    for e in range(E):
        toks = tok_lists[e]
        if len(toks) > C:
            overflow.append((e, toks[C:]))
            toks = toks[:C]
        xe = np.zeros((DIM, C), dtype=BF16)
        xe[:, :len(toks)] = x_flat[toks].T.astype(BF16)
        xp = xe.reshape(ND, 128, C).transpose(1, 0, 2)   # [128, ND, C]
        w1e = np.ascontiguousarray(
            W1[e].reshape(ND, 128, NH, 128).transpose(2, 1, 0, 3).astype(BF16))
        w2e = np.ascontiguousarray(
            W2[e].reshape(NH, 128, ND, 128).transpose(2, 1, 0, 3).astype(BF16))
        in_maps.append({
            "xg0": np.ascontiguousarray(xp[:, :, 0:512]),
            "xg1": np.ascontiguousarray(xp[:, :, 512:PASSES[0]]),
            "xg2": np.ascontiguousarray(xp[:, :, PASSES[0]:C]),
            "w1d": w1e,
            "w2d": w2e,
            "b1d": np.ascontiguousarray(b1[e].reshape(NH, 128).T),
            "b2d": np.ascontiguousarray(b2[e].reshape(ND, 128).T),
        })

    # host FFN for overflow tokens (exact fp32/fp64 -- tiny: capacity is
    # sized so at most a handful of tokens exceed it for this input)
    over_y = {}                                  # (e, tok) -> y row [DIM]
    for e, toks in overflow:
        xo = x_flat[toks].astype(np.float64)
        h = _gelu_exact(xo @ W1[e].astype(np.float64) + b1[e].astype(np.float64))
        yo = h @ W2[e].astype(np.float64) + b2[e].astype(np.float64)
        for i, t in enumerate(toks):
            over_y[(e, t)] = yo[i].astype(np.float32)
    return in_maps, top2, w, pos, over_y


def kernel(x, Wr, W1, b1, W2, b2, _profile=None, _trace_cores=None):
    global _compiled
    from concourse.bass_utils import run_bass_kernel_spmd

    if _compiled is None:
        _compiled = _build()
    nc = _compiled
    in_maps, top2, w, pos, over_y = _prep(x, Wr, W1, b1, W2, b2)
    kwargs = {}
    if _profile:
        kwargs = dict(trace=True, tmpdir=_profile)
        if _trace_cores is not None:
            kwargs["trace_cores"] = _trace_cores
    res = run_bass_kernel_spmd(nc, in_maps, core_ids=list(range(N_CORES)), **kwargs)

    # unshard: per-core y^T [ND, 128, C] -> [C, DIM]; combine top-2 on host
    Y = np.stack([
        np.asarray(res.results[e]["yT"]).reshape(DIM, C).T.astype(np.float32)
        for e in range(E)
    ])                                                             # [E, C, DIM]
    pos_c = np.minimum(pos, C - 1)                 # clamp; overflow rows replaced below
    out = np.zeros((T, DIM), dtype=np.float32)
    for k in range(K):
        yk = Y[top2[:, k], pos_c[:, k]]            # fancy index -> fresh array
        for t in np.where(pos[:, k] >= C)[0]:
            yk[t] = over_y[(top2[t, k], t)]
        out += w[:, k, None] * yk
    out = out.reshape(4, 2048, DIM).astype(np.float32)
    if _profile:
        return out, res
    return out
